# revision 1
# baseline (speedup 1.0000x reference)
"""KAN group-spline kernel for Trainium2 (8 NeuronCores, data-parallel over batch).

Math: out = id_gain[c]*x + spline(clamp(a[c]*x+b[c])) + bias[c], where spline is a
uniform cubic B-spline over K=32 bins with per-group coefficient rows alpha[g].

Device strategy (no gather hardware exists at line rate, so the spline is
evaluated in the "telescoped clamp" basis, which needs no floor/frac/indexing):

    v    = 15.5*a_c*x + 15.5*(b_c+1) + 1          (v in segment S means floor(v)=S)
    F(v) = A_c + sum_{S=0}^{32} g_{c,S}(r_S),      r_S = clamp(v-S, 0, 1)
    g_{c,S}(r) = c1*r + c2*r^2 + c3*r^3            (per channel+segment, host-computed)

  The sum telescopes via spline continuity: below segment 0 / above segment 32 the
  value is exactly the clipped-tap constant, so NO clamp of v is needed at all.

Per 128x4096 tile, per segment S:
  ACT   : q_S = Relu(15.5*x + (oc_c - S))          (scale imm, per-partition bias)
  DVE   : w_S = ((c3*r + c2)*r + c1)*r, r=min(q,1) (ONE custom fused op, 3 per-chan
          scalars via C0/C1/C3-spill)
  GPSIMD: acc += w_S                               (runs parallel to DVE)
Partition dim = (batch,channel) row, so all per-channel params are [P,1] scalars.
"""

import os
import numpy as np

B, C, H, W = 16, 192, 128, 128
K, G = 32, 32
NCORES = 8
SEGS = 33
ROWS = (B // NCORES) * C          # 384 rows per core
FREE = H * W                      # 16384
NCOL = int(os.environ.get("KAN_NCOL", "4096"))
COLT = FREE // NCOL
ROWT = ROWS // 128                # 3

# table column layout (free-dim offsets in the per-rowtile SBUF table)
OFF_IG, OFF_BIAS2, OFF_QB, OFF_C1, OFF_C2, OFF_C3 = 0, 1, 2, 2 + SEGS, 2 + 2 * SEGS, 2 + 3 * SEGS
NTAB = 2 + 4 * SEGS

_BMAT = np.array(
    [
        [1 / 6, -3 / 6, 3 / 6, -1 / 6],
        [4 / 6, 0.0, -6 / 6, 3 / 6],
        [1 / 6, 3 / 6, 3 / 6, -3 / 6],
        [0.0, 0.0, 0.0, 1 / 6],
    ],
    dtype=np.float64,
)  # [tap k, power m]


def build_tables(alpha, a, b, id_gain, bias, group_idx):
    """Host-side exact expansion of the spline into per-(channel,segment) cubic
    coefficients in the telescoped-clamp basis. Returns (scale, tab) where
    tab[rowtile, partition, NTAB] covers rows (batch,channel) = rowtile*128+p."""
    g = group_idx.astype(np.int64)
    alpha_pc = alpha.astype(np.float64)[g]                      # (C, K)
    a64, b64 = a.astype(np.float64), b.astype(np.float64)
    assert np.all(a64 == a64[0]), "fast path needs uniform a (ACT scale is imm)"
    scale = 15.5 * a64[0]
    oc = 15.5 * (b64 + 1.0) + 1.0                               # (C,)

    S = np.arange(SEGS)
    taps = np.clip(S[:, None] - 2 + np.arange(4)[None, :], 0, K - 1)  # (SEGS,4)
    A = alpha_pc[:, taps]                                       # (C, SEGS, 4)
    P = np.einsum("csk,km->csm", A, _BMAT)                      # (C, SEGS, 4)
    c1, c2, c3 = P[..., 1], P[..., 2], P[..., 3]
    Ac = P[:, 0, 0]                                             # value at v=0
    bias2 = bias.astype(np.float64) + Ac

    tab = np.zeros((ROWT, 128, NTAB), dtype=np.float64)
    for t in range(ROWT):
        ch = (t * 128 + np.arange(128)) % C
        tab[t, :, OFF_IG] = id_gain.astype(np.float64)[ch]
        tab[t, :, OFF_BIAS2] = bias2[ch]
        tab[t, :, OFF_QB:OFF_QB + SEGS] = oc[ch, None] - S[None, :]
        tab[t, :, OFF_C1:OFF_C1 + SEGS] = c1[ch]
        tab[t, :, OFF_C2:OFF_C2 + SEGS] = c2[ch]
        tab[t, :, OFF_C3:OFF_C3 + SEGS] = c3[ch]
    return np.float32(scale), tab.astype(np.float32)


def host_emulate(x_rows, scale, tab_t):
    """Numpy emulation of the device program for one row-tile (verification)."""
    q = np.maximum(scale * x_rows[:, None, :] + tab_t[:, OFF_QB:OFF_QB + SEGS, None], 0.0)
    r = np.minimum(q, 1.0)
    c1 = tab_t[:, OFF_C1:OFF_C1 + SEGS, None]
    c2 = tab_t[:, OFF_C2:OFF_C2 + SEGS, None]
    c3 = tab_t[:, OFF_C3:OFF_C3 + SEGS, None]
    w = ((c3 * r + c2) * r + c1) * r
    return x_rows * tab_t[:, OFF_IG, None] + tab_t[:, OFF_BIAS2, None] + w.sum(axis=1)


_PROG_CACHE = {}


def _get_custom_op():
    from concourse.dve_spec import Spec, Src0, C0, C1, C3, One, minn, lower, _spill_c3_to_src1
    from concourse import dve_ops
    from concourse.dve_ops import DveOp, OPS
    from concourse.dve_uop import DveOpSpec

    for op in OPS:
        if op.name == "KAN_SEG":
            return op

    r = minn(Src0, One)
    body = _spill_c3_to_src1(((C3 * r + C1) * r + C0) * r)

    def ref(in0, in1, s0, s1, imm2):
        rr = np.minimum(in0.astype(np.float32), 1.0)
        return ((in1 * rr + s1) * rr + s0) * rr

    spec = Spec(body=body, reference=ref)
    # self-consistent sha: compute what the golden check will compute
    shas = {}
    for ver in ("v3", "v4"):
        tmp = DveOpSpec(name="KAN_SEG", opcode=0, uops=lower(spec, ver=ver), rd1_en=True)
        shas[ver] = tmp.sha(ver)
    op = DveOp("KAN_SEG", spec, subdim=False, uops_sha=shas)
    row = dve_ops._CUSTOM_DVE_ROW_BASE + len(OPS)
    assert row < 0x20
    OPS.append(op)
    dve_ops.CUSTOM_DVE_SPECS[op.name] = spec
    dve_ops._SUB_OPCODE_FOR_NAME[op.name] = row
    assert dve_ops.get_dve_sub_opcode("KAN_SEG") == row
    return op


def _build_program(scale):
    repeat = int(os.environ.get("KAN_REPEAT", "1"))
    key = ("prog", float(scale), NCOL, repeat)
    if key in _PROG_CACHE:
        return _PROG_CACHE[key]

    import concourse.bacc as bacc
    import concourse.mybir as mybir
    from concourse.tile import TileContext
    from concourse.alu_op_type import AluOpType

    kan_op = _get_custom_op()

    nc = bacc.Bacc("TRN2", target_bir_lowering=False, debug=False, num_devices=NCORES)
    x_d = nc.dram_tensor("x", [ROWS, FREE], mybir.dt.float32, kind="ExternalInput").ap()
    tab_d = nc.dram_tensor("tab", [ROWT * 128, NTAB], mybir.dt.float32, kind="ExternalInput").ap()
    out_d = nc.dram_tensor("out", [ROWS, FREE], mybir.dt.float32, kind="ExternalOutput").ap()

    relu = mybir.ActivationFunctionType.Relu

    with TileContext(nc) as tc:
        with (
            tc.tile_pool(name="tabp", bufs=ROWT) as tabp,
            tc.tile_pool(name="xp", bufs=2) as xp,
            tc.tile_pool(name="qp", bufs=int(os.environ.get("KAN_QBUFS", "3"))) as qp,
            tc.tile_pool(name="wp", bufs=3) as wp,
            tc.tile_pool(name="accp", bufs=3) as accp,
        ):
            tabs = []
            for t in range(ROWT):
                tt = tabp.tile([128, NTAB], mybir.dt.float32, tag="tab")
                nc.sync.dma_start(tt[:], tab_d[t * 128:(t + 1) * 128, :])
                tabs.append(tt)

            import contextlib
            loop_ctx = tc.For_i(0, repeat, 1) if repeat > 1 else contextlib.nullcontext()
            with loop_ctx:
                _emit_body(nc, tc, tabs, x_d, out_d, xp, qp, wp, accp, kan_op, scale)

    nc.compile()
    _PROG_CACHE[key] = nc
    return nc


def _emit_body(nc, tc, tabs, x_d, out_d, xp, qp, wp, accp, kan_op, scale):
    import concourse.mybir as mybir
    from concourse.alu_op_type import AluOpType

    relu = mybir.ActivationFunctionType.Relu
    if True:
        if True:
            for t in range(ROWT):
                tt = tabs[t]
                for j in range(COLT):
                    rs, cs = slice(t * 128, (t + 1) * 128), slice(j * NCOL, (j + 1) * NCOL)
                    xt = xp.tile([128, NCOL], mybir.dt.float32, tag="x")
                    nc.sync.dma_start(xt[:], x_d[rs, cs])
                    acc = accp.tile([128, NCOL], mybir.dt.float32, tag="acc")
                    # acc0 = x*ig + bias2  (DVE tensor_scalar, 2 per-partition scalars)
                    nc.vector.tensor_scalar(
                        out=acc[:], in0=xt[:],
                        scalar1=tt[:, OFF_IG:OFF_IG + 1], scalar2=tt[:, OFF_BIAS2:OFF_BIAS2 + 1],
                        op0=AluOpType.mult, op1=AluOpType.add,
                    )
                    for s in range(SEGS):
                        q = qp.tile([128, NCOL], mybir.dt.float32, tag="q")
                        nc.scalar.activation(
                            q[:], xt[:], relu,
                            bias=tt[:, OFF_QB + s:OFF_QB + s + 1], scale=float(scale),
                        )
                        w = wp.tile([128, NCOL], mybir.dt.float32, tag="w")
                        nc.vector._custom_dve(
                            kan_op, out=w[:], in0=q[:],
                            in1=tt[:, OFF_C3 + s:OFF_C3 + s + 1],
                            s0=tt[:, OFF_C1 + s:OFF_C1 + s + 1],
                            s1=tt[:, OFF_C2 + s:OFF_C2 + s + 1],
                        )
                        nxt = accp.tile([128, NCOL], mybir.dt.float32, tag="acc")
                        nc.gpsimd.tensor_tensor(nxt[:], acc[:], w[:], AluOpType.add)
                        acc = nxt
                    nc.sync.dma_start(out_d[rs, cs], acc[:])


def kernel(**inputs):
    x = np.asarray(inputs["x"], dtype=np.float32)
    scale, tab = build_tables(
        np.asarray(inputs["alpha"]), np.asarray(inputs["a"]), np.asarray(inputs["b"]),
        np.asarray(inputs["id_gain"]), np.asarray(inputs["bias"]),
        np.asarray(inputs["group_idx"]),
    )
    from concourse import bass_utils

    nc = _build_program(scale)
    tab_flat = np.ascontiguousarray(tab.reshape(ROWT * 128, NTAB))
    xs = x.reshape(NCORES, B // NCORES, C, H, W)
    in_maps = [
        {"x": np.ascontiguousarray(xs[i].reshape(ROWS, FREE)), "tab": tab_flat}
        for i in range(NCORES)
    ]
    trace = bool(int(os.environ.get("KAN_TRACE", "0")))
    res = bass_utils.run_bass_kernel_spmd(
        nc, in_maps, list(range(NCORES)), trace=trace,
        tmpdir=os.environ.get("KAN_TMPDIR") or None,
    )
    if trace and res.exec_time_ns is not None:
        print(f"HW exec time: {res.exec_time_ns} ns")
    out = np.stack([res.results[i]["out"] for i in range(NCORES)])
    return np.ascontiguousarray(out.reshape(B, C, H, W).astype(np.float32))



# revision 2
# speedup vs baseline: 1.0792x; 1.0792x over previous
"""KAN group-spline kernel for Trainium2 (8 NeuronCores, data-parallel over batch).

Math: out = id_gain[c]*x + F_c(v) + bias[c], where v = 15.5*a*x + oc_c and F_c is
the channel's cubic spline (32-knot uniform B-spline, constant outside [0,33]).

Device basis (exact): truncated-power cubics in the reflected variable
    w     = relu(33 - v) = relu(-15.5*a*x + (33 - oc_c))          [1 ACT pass]
    F_c   = A_c + sum_{j=0}^{32} d_{c,j} * relu(tau_j - w)^3,  tau_j = 33 - j
w reflects v and saturates BOTH tails: v<=0 -> w>=33 -> all hinges 0 -> F=A_c
(true lower plateau); v>=33 -> w=0 -> F = plateau value. d_{c,j} are the f'''
jumps/6 of the true spline -> representation is exact on [0,33].

Per 128-row x NCOL tile:
  ACT : w-pass (Relu, per-partition bias AP, imm scale)
  ACT : acc0 = Identity(ig_p * x + bias2_p)   (per-partition scale+bias APs)
  DVE : 33x fused knot ops (custom STT op): acc += d * relu(tau - w)^3
        in0=w, in1=acc (3D ap -> STT), s0=d per-partition AP, s1=tau literal
Partition dim = (batch,channel) row, so per-channel params are [P,1] scalars.
"""

import os
import numpy as np

B, C, H, W = 16, 192, 128, 128
K, G = 32, 32
NCORES = 8
NSEG = 33                          # spline pieces / knots at v = 0..32
ROWS = (B // NCORES) * C           # 384 rows per core
FREE = H * W                       # 16384
NCOL = int(os.environ.get("KAN_NCOL", "4096"))
COLT = FREE // NCOL
ROWT = ROWS // 128                 # 3

# table column layout (free-dim offsets in the per-rowtile SBUF table)
OFF_WB, OFF_IG, OFF_B2, OFF_D = 0, 1, 2, 3
NTAB = 3 + NSEG

_BMAT = np.array(
    [
        [1 / 6, -3 / 6, 3 / 6, -1 / 6],
        [4 / 6, 0.0, -6 / 6, 3 / 6],
        [1 / 6, 3 / 6, 3 / 6, -3 / 6],
        [0.0, 0.0, 0.0, 1 / 6],
    ],
    dtype=np.float64,
)  # [tap k, power m]

TAUS = [float(33 - j) for j in range(NSEG)]


def build_tables(alpha, a, b, id_gain, bias, group_idx):
    """Exact truncated-power coefficients d (f''' jumps / 6) per channel, plus
    per-channel affine params. Returns (scale, tab[rowtile, 128, NTAB])."""
    g = group_idx.astype(np.int64)
    alpha_pc = alpha.astype(np.float64)[g]                      # (C, K)
    a64, b64 = a.astype(np.float64), b.astype(np.float64)
    assert np.all(a64 == a64[0]), "fast path needs uniform a (ACT scale is imm)"
    scale = 15.5 * a64[0]
    oc = 15.5 * (b64 + 1.0) + 1.0                               # (C,)

    S = np.arange(NSEG)
    taps = np.clip(S[:, None] - 2 + np.arange(4)[None, :], 0, K - 1)  # (NSEG,4)
    A = alpha_pc[:, taps]                                       # (C, NSEG, 4)
    P = np.einsum("csk,km->csm", A, _BMAT)                      # (C, NSEG, 4)
    c3 = P[..., 3]                                              # t^3 coeff per piece
    d = np.diff(c3, axis=1, prepend=0.0)                        # (C, NSEG) hinge coeffs
    Ac = P[:, 0, 0]                                             # value at v=0
    bias2 = bias.astype(np.float64) + Ac

    tab = np.zeros((ROWT, 128, NTAB), dtype=np.float64)
    for t in range(ROWT):
        ch = (t * 128 + np.arange(128)) % C
        tab[t, :, OFF_WB] = 33.0 - oc[ch]
        tab[t, :, OFF_IG] = id_gain.astype(np.float64)[ch]
        tab[t, :, OFF_B2] = bias2[ch]
        tab[t, :, OFF_D:OFF_D + NSEG] = d[ch]                   # d_j for tau_j=33-j
    return np.float32(scale), tab.astype(np.float32)


def host_emulate(x_rows, scale, tab_t):
    """Numpy fp32 emulation of the device program for one row-tile."""
    f = np.float32
    w = np.maximum(tab_t[:, OFF_WB, None] - f(scale) * x_rows, f(0))
    acc = tab_t[:, OFF_IG, None] * x_rows + tab_t[:, OFF_B2, None]
    for j, tau in enumerate(TAUS):
        q = np.maximum(f(tau) - w, f(0))
        acc = acc + tab_t[:, OFF_D + j, None] * (q * q) * q
    return acc


_PROG_CACHE = {}


def _get_custom_op():
    from concourse.dve_spec import Spec, Src0, Src1, C0, C1, relu, sq, lower
    from concourse import dve_ops
    from concourse.dve_ops import DveOp, OPS
    from concourse.dve_uop import DveOpSpec

    for op in OPS:
        if op.name == "KAN_CUB":
            return op

    q = relu(C1 - Src0)
    body = Src1 + (C0 * sq(q)) * q

    def ref(in0, in1, s0, s1, imm2):
        qq = np.maximum(np.float32(s1) - in0, np.float32(0)).astype(np.float32)
        return in1 + (s0 * (qq * qq)) * qq

    spec = Spec(body=body, reference=ref)
    shas = {}
    for ver in ("v3", "v4"):
        tmp = DveOpSpec(name="KAN_CUB", opcode=0, uops=lower(spec, ver=ver), rd1_en=True)
        shas[ver] = tmp.sha(ver)
    op = DveOp("KAN_CUB", spec, subdim=False, uops_sha=shas)
    row = dve_ops._CUSTOM_DVE_ROW_BASE + len(OPS)
    assert row < 0x20
    OPS.append(op)
    dve_ops.CUSTOM_DVE_SPECS[op.name] = spec
    dve_ops._SUB_OPCODE_FOR_NAME[op.name] = row
    assert dve_ops.get_dve_sub_opcode("KAN_CUB") == row
    return op


def _build_program(scale):
    repeat = int(os.environ.get("KAN_REPEAT", "1"))
    key = ("prog", float(scale), NCOL, repeat)
    if key in _PROG_CACHE:
        return _PROG_CACHE[key]

    import concourse.bacc as bacc
    import concourse.mybir as mybir
    from concourse.tile import TileContext

    kan_op = _get_custom_op()

    nc = bacc.Bacc("TRN2", target_bir_lowering=False, debug=False, num_devices=NCORES)
    x_d = nc.dram_tensor("x", [ROWS, FREE], mybir.dt.float32, kind="ExternalInput").ap()
    tab_d = nc.dram_tensor("tab", [ROWT * 128, NTAB], mybir.dt.float32, kind="ExternalInput").ap()
    out_d = nc.dram_tensor("out", [ROWS, FREE], mybir.dt.float32, kind="ExternalOutput").ap()

    with TileContext(nc) as tc:
        with (
            tc.tile_pool(name="tabp", bufs=ROWT) as tabp,
            tc.tile_pool(name="xp", bufs=2) as xp,
            tc.tile_pool(name="wp", bufs=2) as wp,
            tc.tile_pool(name="accp", bufs=int(os.environ.get("KAN_ACCBUFS", "4"))) as accp,
        ):
            tabs = []
            for t in range(ROWT):
                tt = tabp.tile([128, NTAB], mybir.dt.float32, tag="tab")
                nc.sync.dma_start(tt[:], tab_d[t * 128:(t + 1) * 128, :])
                tabs.append(tt)

            import contextlib
            loop_ctx = tc.For_i(0, repeat, 1) if repeat > 1 else contextlib.nullcontext()
            with loop_ctx:
                _emit_body(nc, tc, tabs, x_d, out_d, xp, wp, accp, kan_op, scale)

    nc.compile()
    _PROG_CACHE[key] = nc
    return nc


def _emit_body(nc, tc, tabs, x_d, out_d, xp, wp, accp, kan_op, scale):
    import concourse.mybir as mybir

    relu_f = mybir.ActivationFunctionType.Relu
    ident = mybir.ActivationFunctionType.Identity
    for t in range(ROWT):
        tt = tabs[t]
        for j in range(COLT):
            rs, cs = slice(t * 128, (t + 1) * 128), slice(j * NCOL, (j + 1) * NCOL)
            xt = xp.tile([128, NCOL], mybir.dt.float32, tag="x")
            nc.sync.dma_start(xt[:], x_d[rs, cs])
            wt = wp.tile([128, NCOL], mybir.dt.float32, tag="w")
            nc.scalar.activation(
                wt[:], xt[:], relu_f,
                bias=tt[:, OFF_WB:OFF_WB + 1], scale=-float(scale),
            )
            acc = accp.tile([128, NCOL, 1], mybir.dt.float32, tag="acc")
            nc.scalar.activation(
                acc[:, :, 0], xt[:], ident,
                bias=tt[:, OFF_B2:OFF_B2 + 1], scale=tt[:, OFF_IG:OFF_IG + 1],
            )
            for s in range(NSEG):
                nxt = accp.tile([128, NCOL, 1], mybir.dt.float32, tag="acc")
                nc.vector._custom_dve(
                    kan_op, out=nxt[:, :, 0], in0=wt[:], in1=acc[:],
                    s0=tt[:, OFF_D + s:OFF_D + s + 1], s1=TAUS[s],
                )
                acc = nxt
            nc.sync.dma_start(out_d[rs, cs], acc[:, :, 0])


def kernel(**inputs):
    x = np.asarray(inputs["x"], dtype=np.float32)
    scale, tab = build_tables(
        np.asarray(inputs["alpha"]), np.asarray(inputs["a"]), np.asarray(inputs["b"]),
        np.asarray(inputs["id_gain"]), np.asarray(inputs["bias"]),
        np.asarray(inputs["group_idx"]),
    )
    from concourse import bass_utils

    nc = _build_program(scale)
    tab_flat = np.ascontiguousarray(tab.reshape(ROWT * 128, NTAB))
    xs = x.reshape(NCORES, B // NCORES, C, H, W)
    in_maps = [
        {"x": np.ascontiguousarray(xs[i].reshape(ROWS, FREE)), "tab": tab_flat}
        for i in range(NCORES)
    ]
    trace = bool(int(os.environ.get("KAN_TRACE", "0")))
    res = bass_utils.run_bass_kernel_spmd(
        nc, in_maps, list(range(NCORES)), trace=trace,
        tmpdir=os.environ.get("KAN_TMPDIR") or None,
    )
    if trace and res.exec_time_ns is not None:
        print(f"HW exec time: {res.exec_time_ns} ns")
    out = np.stack([res.results[i]["out"] for i in range(NCORES)])
    return np.ascontiguousarray(out.reshape(B, C, H, W).astype(np.float32))


# revision 3
# speedup vs baseline: 1.5854x; 1.4691x over previous
"""KAN group-spline kernel for Trainium2 (8 NeuronCores, data-parallel over batch).

Math: out = id_gain[c]*x + F_c(v) + bias[c], v = 15.5*a*x + oc_c, F_c = channel's
cubic spline (32-knot uniform B-spline, constant outside v in [0,33]).

Exact device basis: truncated-power cubics in the reflected variable
    w   = relu(33 - v) = relu(-15.5*a*x + (33 - oc_c))            [ACT, 1 pass]
    F_c = A_c + sum_{j=0}^{32} d_{c,j} * relu(tau_j - w)^3,  tau_j = 33 - j
w saturates BOTH tails exactly (v<=0 -> all hinges 0; v>=33 -> w=0 plateau).
d_{c,j} = f''' jumps / 6 of the true spline -> exact representation.

Engine plan per 128xNCOL tile (all knot ops independent -> full pipelining):
  ACT : w = Relu(-scale*x + (33-oc))             [per-partition bias AP]
  PE  : psum  = diag(ig) @ x_chunk               [start]  \   init
        psum += diag(bias+A) @ ones              [accum]  /   (acc0)
  DVE : y_j = d_j * relu(tau_j - w)^3            [custom TTSS op, 1-tensor ->
                                                  full rate; s0=d AP, s1=tau imm]
  PE  : psum += I @ y_j_chunk                    [identity diag accumulate]
  ACT : out_sbuf = Copy(psum)                    [evacuate]
DVE never streams two SBUF tensors (the S2S2D2_STT 2-source half-rate trap).
"""

import os
import numpy as np

B, C, H, W = 16, 192, 128, 128
K, G = 32, 32
NCORES = 8
NSEG = 33                          # spline pieces / knots at v = 0..32
ROWS = (B // NCORES) * C           # 384 rows per core
FREE = H * W                       # 16384
NCOL = int(os.environ.get("KAN_NCOL", "4096"))
COLT = FREE // NCOL
ROWT = ROWS // 128                 # 3
MMF = 512                          # fp32 matmul max free size / PSUM bank
NMM = NCOL // MMF

OFF_WB, OFF_D = 0, 1
NTAB = 1 + NSEG
NWTS = 1 + 2 * ROWT                # identity + per-rowtile {ig, bias2} diags

_BMAT = np.array(
    [
        [1 / 6, -3 / 6, 3 / 6, -1 / 6],
        [4 / 6, 0.0, -6 / 6, 3 / 6],
        [1 / 6, 3 / 6, 3 / 6, -3 / 6],
        [0.0, 0.0, 0.0, 1 / 6],
    ],
    dtype=np.float64,
)  # [tap k, power m]

TAUS = [float(33 - j) for j in range(NSEG)]


def build_tables(alpha, a, b, id_gain, bias, group_idx):
    """Exact truncated-power coefficients d (f''' jumps / 6) per channel plus
    per-channel affine params. Returns (scale, tab[ROWT,128,NTAB],
    wts[NWTS,128,128])."""
    g = group_idx.astype(np.int64)
    alpha_pc = alpha.astype(np.float64)[g]                      # (C, K)
    a64, b64 = a.astype(np.float64), b.astype(np.float64)
    assert np.all(a64 == a64[0]), "fast path needs uniform a (ACT scale is imm)"
    scale = 15.5 * a64[0]
    oc = 15.5 * (b64 + 1.0) + 1.0                               # (C,)

    S = np.arange(NSEG)
    taps = np.clip(S[:, None] - 2 + np.arange(4)[None, :], 0, K - 1)
    A = alpha_pc[:, taps]                                       # (C, NSEG, 4)
    P = np.einsum("csk,km->csm", A, _BMAT)                      # (C, NSEG, 4)
    c3 = P[..., 3]
    d = np.diff(c3, axis=1, prepend=0.0)                        # (C, NSEG)
    Ac = P[:, 0, 0]
    bias2 = bias.astype(np.float64) + Ac

    tab = np.zeros((ROWT, 128, NTAB), dtype=np.float64)
    wts = np.zeros((NWTS, 128, 128), dtype=np.float64)
    wts[0] = np.eye(128)
    for t in range(ROWT):
        ch = (t * 128 + np.arange(128)) % C
        tab[t, :, OFF_WB] = 33.0 - oc[ch]
        tab[t, :, OFF_D:OFF_D + NSEG] = d[ch]
        wts[1 + 2 * t] = np.diag(id_gain.astype(np.float64)[ch])
        wts[2 + 2 * t] = np.diag(bias2[ch])
    return np.float32(scale), tab.astype(np.float32), wts.astype(np.float32)


def host_emulate(x_rows, scale, tab_t, ig_diag, b2_diag):
    """Numpy fp32 emulation of the device program for one row-tile."""
    f = np.float32
    w = np.maximum(tab_t[:, OFF_WB, None] - f(scale) * x_rows, f(0))
    acc = np.diag(ig_diag)[:, None] * x_rows + np.diag(b2_diag)[:, None]
    for j, tau in enumerate(TAUS):
        q = np.maximum(f(tau) - w, f(0))
        acc = acc + tab_t[:, OFF_D + j, None] * ((q * q) * q)
    return acc


_PROG_CACHE = {}


def _get_custom_op():
    from concourse.dve_spec import Spec, Src0, C0, C1, relu, sq, lower
    from concourse import dve_ops
    from concourse.dve_ops import DveOp, OPS
    from concourse.dve_uop import DveOpSpec

    for op in OPS:
        if op.name == "KAN_CUB":
            return op

    q = relu(C1 - Src0)
    body = (C0 * sq(q)) * q

    def ref(in0, in1, s0, s1, imm2):
        qq = np.maximum(np.float32(s1) - in0, np.float32(0)).astype(np.float32)
        return (s0 * (qq * qq)) * qq

    spec = Spec(body=body, reference=ref)
    shas = {}
    for ver in ("v3", "v4"):
        tmp = DveOpSpec(name="KAN_CUB", opcode=0, uops=lower(spec, ver=ver), rd1_en=False)
        shas[ver] = tmp.sha(ver)
    op = DveOp("KAN_CUB", spec, subdim=False, uops_sha=shas)
    row = dve_ops._CUSTOM_DVE_ROW_BASE + len(OPS)
    assert row < 0x20
    OPS.append(op)
    dve_ops.CUSTOM_DVE_SPECS[op.name] = spec
    dve_ops._SUB_OPCODE_FOR_NAME[op.name] = row
    assert dve_ops.get_dve_sub_opcode("KAN_CUB") == row
    return op


def _build_program(scale):
    repeat = int(os.environ.get("KAN_REPEAT", "1"))
    key = ("prog", float(scale), NCOL, repeat)
    if key in _PROG_CACHE:
        return _PROG_CACHE[key]

    import concourse.bacc as bacc
    import concourse.mybir as mybir
    from concourse.tile import TileContext

    kan_op = _get_custom_op()

    nc = bacc.Bacc("TRN2", target_bir_lowering=False, debug=False, num_devices=NCORES)
    x_d = nc.dram_tensor("x", [ROWS, FREE], mybir.dt.float32, kind="ExternalInput").ap()
    tab_d = nc.dram_tensor("tab", [ROWT * 128, NTAB], mybir.dt.float32, kind="ExternalInput").ap()
    wts_d = nc.dram_tensor("wts", [NWTS * 128, 128], mybir.dt.float32, kind="ExternalInput").ap()
    out_d = nc.dram_tensor("out", [ROWS, FREE], mybir.dt.float32, kind="ExternalOutput").ap()

    with TileContext(nc) as tc:
        with (
            tc.tile_pool(name="tabp", bufs=ROWT) as tabp,
            tc.tile_pool(name="wtsp", bufs=NWTS) as wtsp,
            tc.tile_pool(name="onesp", bufs=1) as onesp,
            tc.tile_pool(name="xp", bufs=2) as xp,
            tc.tile_pool(name="wp", bufs=2) as wp,
            tc.tile_pool(name="yp", bufs=int(os.environ.get("KAN_YBUFS", "4"))) as yp,
            tc.tile_pool(name="outp", bufs=2) as outp,
            tc.tile_pool(name="psp", bufs=1, space="PSUM") as psp,
        ):
            tabs, wtss = [], []
            for t in range(ROWT):
                tt = tabp.tile([128, NTAB], mybir.dt.float32, tag="tab")
                nc.sync.dma_start(tt[:], tab_d[t * 128:(t + 1) * 128, :])
                tabs.append(tt)
            for i in range(NWTS):
                wt_ = wtsp.tile([128, 128], mybir.dt.float32, tag="wts")
                nc.sync.dma_start(wt_[:], wts_d[i * 128:(i + 1) * 128, :])
                wtss.append(wt_)
            ones = onesp.tile([128, MMF], mybir.dt.float32, tag="ones")
            nc.vector.memset(ones[:], 1.0)

            import contextlib
            loop_ctx = tc.For_i(0, repeat, 1) if repeat > 1 else contextlib.nullcontext()
            with loop_ctx:
                _emit_body(nc, tc, tabs, wtss, ones, x_d, out_d, xp, wp, yp, outp, psp, kan_op, scale)

    nc.compile()
    _PROG_CACHE[key] = nc
    return nc


def _emit_body(nc, tc, tabs, wtss, ones, x_d, out_d, xp, wp, yp, outp, psp, kan_op, scale):
    import concourse.mybir as mybir

    relu_f = mybir.ActivationFunctionType.Relu
    copy_f = mybir.ActivationFunctionType.Copy
    ident = wtss[0]
    for t in range(ROWT):
        tt = tabs[t]
        igd, b2d = wtss[1 + 2 * t], wtss[2 + 2 * t]
        for j in range(COLT):
            rs, cs = slice(t * 128, (t + 1) * 128), slice(j * NCOL, (j + 1) * NCOL)
            xt = xp.tile([128, NCOL], mybir.dt.float32, tag="x")
            nc.sync.dma_start(xt[:], x_d[rs, cs])
            wt = wp.tile([128, NCOL], mybir.dt.float32, tag="w")
            nc.scalar.activation(
                wt[:], xt[:], relu_f,
                bias=tt[:, OFF_WB:OFF_WB + 1], scale=-float(scale),
            )
            ps = psp.tile([128, NCOL], mybir.dt.float32, tag="ps")
            for m in range(NMM):
                ms = slice(m * MMF, (m + 1) * MMF)
                nc.tensor.matmul(ps[:, ms], igd[:], xt[:, ms], start=True, stop=False)
                nc.tensor.matmul(ps[:, ms], b2d[:], ones[:], start=False, stop=False)
            for s in range(NSEG):
                y = yp.tile([128, NCOL], mybir.dt.float32, tag="y")
                nc.vector._custom_dve(
                    kan_op, out=y[:], in0=wt[:],
                    s0=tt[:, OFF_D + s:OFF_D + s + 1], s1=TAUS[s],
                )
                last = s == NSEG - 1
                for m in range(NMM):
                    ms = slice(m * MMF, (m + 1) * MMF)
                    nc.tensor.matmul(ps[:, ms], ident[:], y[:, ms], start=False, stop=last)
            outt = outp.tile([128, NCOL], mybir.dt.float32, tag="out")
            nc.scalar.activation(outt[:], ps[:], copy_f, bias=0.0)
            nc.sync.dma_start(out_d[rs, cs], outt[:])


def kernel(**inputs):
    x = np.asarray(inputs["x"], dtype=np.float32)
    scale, tab, wts = build_tables(
        np.asarray(inputs["alpha"]), np.asarray(inputs["a"]), np.asarray(inputs["b"]),
        np.asarray(inputs["id_gain"]), np.asarray(inputs["bias"]),
        np.asarray(inputs["group_idx"]),
    )
    from concourse import bass_utils

    nc = _build_program(scale)
    tab_flat = np.ascontiguousarray(tab.reshape(ROWT * 128, NTAB))
    wts_flat = np.ascontiguousarray(wts.reshape(NWTS * 128, 128))
    xs = x.reshape(NCORES, B // NCORES, C, H, W)
    in_maps = [
        {"x": np.ascontiguousarray(xs[i].reshape(ROWS, FREE)), "tab": tab_flat,
         "wts": wts_flat}
        for i in range(NCORES)
    ]
    trace = bool(int(os.environ.get("KAN_TRACE", "0")))
    res = bass_utils.run_bass_kernel_spmd(
        nc, in_maps, list(range(NCORES)), trace=trace,
        tmpdir=os.environ.get("KAN_TMPDIR") or None,
    )
    if trace and res.exec_time_ns is not None:
        print(f"HW exec time: {res.exec_time_ns} ns")
    out = np.stack([res.results[i]["out"] for i in range(NCORES)])
    return np.ascontiguousarray(out.reshape(B, C, H, W).astype(np.float32))
